# revision 10
# baseline (speedup 1.0000x reference)
"""Block-diagonal masked dense + BatchNorm(train) + ReLU on 8 TRN2 NeuronCores.

Math: out = x @ (W * blockdiag_mask) + bias; BN over batch; relu.
The mask keeps 64 diagonal blocks of shape [64 in, 64 out]. Group g only
couples x[:, 64g:64g+64] to out[:, 64g:64g+64].

Sharding: groups are split across cores (8 groups per core). Each core owns a
disjoint 512-column slice of both input and output features, so the matmul and
the per-feature batch statistics are fully core-local (no collectives).

Per-core device program (all shapes hardcoded):
  inputs:  xT [512, 4096] (x slice transposed on host, pre-rounded to
           float32r), wd [512, 128] (per 128-row chunk a 2x2 block-diagonal
           of two 64x64 group blocks, pre-rounded), gm/bt [512]
  output:  yT [512, 4096] (y slice transposed; host transposes back)
  phase 1: for each 128-row chunk c (2 groups) and batch tile t (512):
           psum[j, b] = W2_c^T xT_c via one K=128 f32r matmul (the
           block-diagonal zeros kill cross-group terms, f32r streams at
           1 cycle/row vs fp32's ~4); bn_stats/bn_aggr give mean/var
           per output feature.
  coefs:   A = gamma * rsqrt(var + eps); B = beta - mean * A.
           (bias cancels in BN: out and mean(out) shift equally, and variance
           is bias-invariant, so bias never needs to reach the device.)
  phase 2: recompute the matmul (x stays SBUF-resident; PE is cheap) and
           apply relu(psum * A + B) in one ScalarE pass, PSUM -> SBUF ->
           DRAM. Phase-1/phase-2 chunks are interleaved so DVE (stats),
           ACT (relu), input DMA and output DMA all stream concurrently.

Accuracy: ~1.5e-4 rel L2 vs the fp32 reference, dominated by the float32r
11-bit-mantissa input rounding (the f32r matmul itself is exact on
pre-rounded inputs; BN math runs in fp32).
"""

import numpy as np

import concourse.bass as bass
import concourse.tile as tile
from concourse import mybir
from concourse.bass_utils import run_bass_kernel_spmd

F32 = mybir.dt.float32

NCORES = 8
BATCH = 4096
DIM = 4096
DCORE = DIM // NCORES          # 512 features per core
CHUNKS = DCORE // 128          # 4 partition chunks (2 groups each)
BTILE = 512                    # batch tile (one PSUM bank, fp32 moving max)
BTILES = BATCH // BTILE        # 8
EPS = 1e-3

_MAX_WAITS = 1


def _split_multi_waits(nc: bass.Bass, max_waits: int = _MAX_WAITS) -> None:
    # The walrus build in this container rejects instructions carrying more
    # than one sync-wait command (any engine, any opcode). Hoist extra waits
    # onto same-engine NOPs inserted immediately before the instruction —
    # identical semantics, since the engine blocks on each wait in order.
    # Snapshot every block BEFORE creating any nop: the engine builders append
    # new instructions to the current (last) block as a side effect, and the
    # final wholesale reassignment below discards those spurious appends.
    snapshots = [
        (bb, list(bb.instructions)) for f in nc.m.functions for bb in f.blocks
    ]
    rebuilt = []
    for bb, insts in snapshots:
        new = []
        for ins in insts:
            si = getattr(ins, "sync_info", None)
            waits = list(si.on_wait) if si is not None and si.on_wait else []
            if len(waits) > max_waits:
                head = waits[:-max_waits]
                for i in range(0, len(head), max_waits):
                    nop = nc.engines[ins.engine].nop().ins
                    nop.sync_info = mybir.SyncInfo(
                        on_wait=head[i : i + max_waits], on_update=[]
                    )
                    new.append(nop)
                ins.sync_info = mybir.SyncInfo(
                    on_wait=waits[-max_waits:],
                    on_update=list(si.on_update or []),
                )
            new.append(ins)
        rebuilt.append((bb, new))
    for bb, new in rebuilt:
        bb.instructions = new


BF16 = mybir.dt.bfloat16
MEGA = 1024                    # PSUM mega-tile free dim (2 banks, 2 matmuls)
MEGAS = BATCH // MEGA          # 4 mega tiles per chunk per phase


def _build_nc() -> bass.Bass:
    nc = bass.Bass()
    # x, the diagonal weight blocks, and the output all move as bf16: the
    # kernel is HBM-bound, so halving I/O bytes halves exec time, and the
    # 2e-2 rel-err gate leaves ~10x headroom over bf16's ~2e-3. Matmul
    # accumulates in fp32 PSUM; BN math stays fp32 end-to-end.
    xT = nc.dram_tensor("xT", [DCORE, BATCH], BF16, kind="ExternalInput")
    wd = nc.dram_tensor("wd", [DCORE, 128], BF16, kind="ExternalInput")
    gm = nc.dram_tensor("gm", [DCORE], F32, kind="ExternalInput")
    bt = nc.dram_tensor("bt", [DCORE], F32, kind="ExternalInput")
    yT = nc.dram_tensor("yT", [DCORE, BATCH], BF16, kind="ExternalOutput")

    with tile.TileContext(nc) as tc:
        with (
            tc.tile_pool(name="singles", bufs=1) as singles,
            tc.tile_pool(name="stats", bufs=1) as statp,
            tc.tile_pool(name="psum", bufs=4, space="PSUM") as psum,
            tc.tile_pool(name="y", bufs=2) as ypool,
        ):
            # Input stream (weights + x) issues on the Scalar HWDGE queue,
            # output stream on the Sync HWDGE queue: two independent
            # descriptor pipelines instead of one serialized ring.
            wsb = singles.tile([128, CHUNKS, 128], BF16)
            nc.scalar.dma_start(
                wsb[:], wd.rearrange("(c p) m -> p c m", p=128)
            )

            # Resident x. Partition p of chunk c holds feature c*128+p.
            # Chunk 0 lands in quarters so the first matmul starts ASAP;
            # later chunks as whole 1 MB transfers (8 KB/partition lines).
            xsb = singles.tile([128, CHUNKS, BATCH], BF16)
            xTv = xT.rearrange("(c p) b -> p c b", p=128)
            for h in range(4):
                sl = bass.ds(h * (BATCH // 4), BATCH // 4)
                nc.scalar.dma_start(xsb[:, 0, sl], xTv[:, 0, sl])
            for h in range(2):
                sl = bass.ds(h * (BATCH // 2), BATCH // 2)
                nc.scalar.dma_start(xsb[:, 1, sl], xTv[:, 1, sl])
            for c in range(2, CHUNKS):
                nc.scalar.dma_start(xsb[:, c, :], xTv[:, c, :])

            gsb = singles.tile([128, CHUNKS], F32)
            nc.sync.dma_start(gsb[:], gm.rearrange("(c p) -> p c", p=128))
            bsb = singles.tile([128, CHUNKS], F32)
            nc.sync.dma_start(bsb[:], bt.rearrange("(c p) -> p c", p=128))
            epsb = singles.tile([128, 1], F32)
            nc.vector.memset(epsb[:], EPS)

            stats = statp.tile([128, CHUNKS, BTILES, 6], F32)
            mv = statp.tile([128, CHUNKS, 2], F32)
            coefA = statp.tile([128, CHUNKS], F32)
            coefB = statp.tile([128, CHUNKS], F32)
            tmp = statp.tile([128, CHUNKS], F32)

            yTv = yT.rearrange("(c p) b -> p c b", p=128)

            def chunk(c: int):
                # One chunk's whole batch fills all 8 PSUM banks (4 mega
                # tiles of 2 banks), so the matmul runs ONCE: stats read
                # PSUM, then relu drains PSUM directly. The next chunk's
                # matmuls recycle each mega as soon as its relu is done.
                pss = []
                for m in range(MEGAS):
                    ps = psum.tile([128, MEGA], F32, tag="ps")
                    for q in range(MEGA // BTILE):
                        nc.tensor.matmul(
                            ps[:, bass.ds(q * BTILE, BTILE)],
                            lhsT=wsb[:, c, :],
                            rhs=xsb[:, c, bass.ds(m * MEGA + q * BTILE, BTILE)],
                            start=True, stop=True,
                        )
                        # bn_stats is HW-capped at 512 free elems
                        nc.vector.bn_stats(
                            stats[:, c, m * (MEGA // BTILE) + q, :],
                            ps[:, bass.ds(q * BTILE, BTILE)],
                        )
                    pss.append(ps)
                nc.vector.bn_aggr(mv[:, c, :], stats[:, c, :, :])
                nc.scalar.activation(
                    tmp[:, c : c + 1], mv[:, c, 1:2],
                    mybir.ActivationFunctionType.Sqrt,
                    bias=epsb[:], scale=1.0,
                )
                nc.vector.reciprocal(tmp[:, c : c + 1], tmp[:, c : c + 1])
                nc.vector.tensor_mul(
                    coefA[:, c : c + 1], tmp[:, c : c + 1], gsb[:, c : c + 1]
                )
                nc.vector.tensor_mul(
                    tmp[:, c : c + 1], mv[:, c, 0:1], coefA[:, c : c + 1]
                )
                nc.vector.tensor_sub(
                    coefB[:, c : c + 1], bsb[:, c : c + 1], tmp[:, c : c + 1]
                )
                yt = ypool.tile([128, BATCH], BF16, tag="yt")
                for m in range(MEGAS):
                    nc.scalar.activation(
                        yt[:, bass.ds(m * MEGA, MEGA)], pss[m][:],
                        mybir.ActivationFunctionType.Relu,
                        bias=coefB[:, c : c + 1], scale=coefA[:, c : c + 1],
                    )
                    if m == 1:
                        nc.sync.dma_start(
                            yTv[:, c, 0 : 2 * MEGA], yt[:, 0 : 2 * MEGA]
                        )
                nc.sync.dma_start(
                    yTv[:, c, 2 * MEGA : BATCH], yt[:, 2 * MEGA : BATCH]
                )

            for c in range(CHUNKS):
                chunk(c)
    _split_multi_waits(nc)
    return nc


_NC_CACHE: bass.Bass | None = None


def _get_nc() -> bass.Bass:
    global _NC_CACHE
    if _NC_CACHE is None:
        _NC_CACHE = _build_nc()
    return _NC_CACHE


from ml_dtypes import bfloat16 as _bf16


def _make_in_maps(x, weight, gamma, beta):
    in_maps = []
    for c in range(NCORES):
        sl = slice(c * DCORE, (c + 1) * DCORE)
        xT = np.ascontiguousarray(x[:, sl].T).astype(_bf16)
        # Per 128-row chunk: [[w_{2c}, 0], [0, w_{2c+1}]] block-diagonal.
        wdc = np.zeros((DCORE, 128), np.float32)
        for g in range(DCORE // 64):
            r = slice(c * DCORE + g * 64, c * DCORE + (g + 1) * 64)
            col = (g % 2) * 64
            wdc[g * 64 : (g + 1) * 64, col : col + 64] = weight[r, r]
        in_maps.append(
            {
                "xT": xT,
                "wd": wdc.astype(_bf16),
                "gm": np.ascontiguousarray(gamma[sl]),
                "bt": np.ascontiguousarray(beta[sl]),
            }
        )
    return in_maps


def kernel(x, weight, bias, gamma, beta, **_run_kwargs) -> np.ndarray:
    x = np.asarray(x, np.float32)
    weight = np.asarray(weight, np.float32)
    gamma = np.asarray(gamma, np.float32)
    beta = np.asarray(beta, np.float32)
    # bias is algebraically irrelevant: BN subtracts the batch mean, which
    # absorbs any constant per-feature shift, and variance is shift-invariant.

    nc = _get_nc()
    res = run_bass_kernel_spmd(
        nc, _make_in_maps(x, weight, gamma, beta),
        core_ids=list(range(NCORES)), **_run_kwargs,
    )
    out = np.empty((BATCH, DIM), np.float32)
    for c, r in enumerate(res.results):
        out[:, c * DCORE : (c + 1) * DCORE] = r["yT"].T.astype(np.float32)
    kernel.last_results = res
    return out

